# revision 2
# baseline (speedup 1.0000x reference)
"""AdjMatrixGenerator Trainium2 kernel.

Reference computation (B=16, N=256, F=64, H=64):
    a = h @ w1a.T ; c = h @ w1b.T            # [B,N,H] each (w1 split in half)
    z = relu(a[:,i,None,:] + c[:,None,j,:] + b1)   # [B,N,N,H]
    adj = sigmoid(z @ w2.T + b2)             # [B,N,N]
    diagonal forced to 1.

Sharding: data-parallel over batch, 2 batches per core x 8 cores.

Per-core device algorithm:
  - hT [64,512] (host-pretransposed shard) -> PE matmuls produce
    aT2/cT2 [128,256] where partitions = (batch,h) stacked pairs and
    free = node index within batch.
  - For each node i (a "duo" covers the same i in both local batches):
      z2[p, j] = relu(cT2[p, j] + aT2[p, i])   (DVE tensor_scalar add+max,
                                                ACT activation Relu for a
                                                subset - both engines run in
                                                parallel)
      one PE matmul reduces over h with w2 using a shifted-window weight
      matrix so duo d accumulates into PSUM partitions 2d/2d+1 of a dense
      [128,256] tile (64 duos per PSUM tile).
  - ACT sigmoid (+b2) on each accumulated PSUM tile -> DMA to DRAM.
Output rows are (i-major, batch-interleaved); host reorders + sets diag=1.
"""

import sys

for _p in ("/opt/trn_rl_repo",):
    if _p not in sys.path:
        sys.path.insert(0, _p)

import numpy as np
import ml_dtypes

import concourse.bass as bass
import concourse.tile as tile
from concourse import bacc, mybir
from concourse.bass_utils import run_bass_kernel_spmd

B, N, F, H = 16, 256, 64, 64
NCORES = 8
BLOC = B // NCORES          # batches per core = 2
IL = BLOC * N               # local node-rows = 512
NG = 4                      # PSUM groups per core
DUOS_PER_G = N // NG        # 64 duos per group
ACT_STRIDE = 4              # every 4th duo's relu runs on ScalarE

F32 = mybir.dt.float32
BF16 = mybir.dt.bfloat16

_COMPILED = None


def _build():
    nc = bacc.Bacc("TRN2", target_bir_lowering=False, debug=False,
                   enable_asserts=False, num_devices=NCORES)

    hT_d = nc.dram_tensor("hT", [F, IL], F32, kind="ExternalInput").ap()
    w1aT_d = nc.dram_tensor("w1aT", [F, H], F32, kind="ExternalInput").ap()
    w1bT_d = nc.dram_tensor("w1bT", [F, H], F32, kind="ExternalInput").ap()
    b1v_d = nc.dram_tensor("b1v", [2 * H, 1], F32, kind="ExternalInput").ap()
    b2v_d = nc.dram_tensor("b2v", [2 * H, 1], F32, kind="ExternalInput").ap()
    wbig_d = nc.dram_tensor("wbig", [128, 256], BF16, kind="ExternalInput").ap()
    out_d = nc.dram_tensor("out", [2 * N, N], F32, kind="ExternalOutput").ap()

    Relu = mybir.ActivationFunctionType.Relu
    Sigmoid = mybir.ActivationFunctionType.Sigmoid
    ADD = mybir.AluOpType.add
    MAX = mybir.AluOpType.max

    with tile.TileContext(nc) as tc:
        with (
            tc.tile_pool(name="const", bufs=1) as cpool,
            tc.tile_pool(name="z", bufs=8) as zpool,
            tc.tile_pool(name="sig", bufs=2) as spool,
            tc.tile_pool(name="pconst", bufs=1, space=bass.MemorySpace.PSUM) as ppc,
            tc.tile_pool(name="pmain", bufs=2, space=bass.MemorySpace.PSUM) as ppm,
        ):
            hT = cpool.tile([F, IL], F32)
            w1aT = cpool.tile([F, H], F32)
            w1bT = cpool.tile([F, H], F32)
            b1v = cpool.tile([2 * H, 1], F32)
            b2v = cpool.tile([2 * H, 1], F32)
            wbig = cpool.tile([128, 256], BF16)
            nc.sync.dma_start(hT[:], hT_d)
            nc.sync.dma_start(w1aT[:], w1aT_d)
            nc.sync.dma_start(w1bT[:], w1bT_d)
            nc.sync.dma_start(b1v[:], b1v_d)
            nc.sync.dma_start(b2v[:], b2v_d)
            nc.sync.dma_start(wbig[:], wbig_d)

            # aT2 / cT2: [128, 256]; partition p = (batch, h), free = node i.
            psum_a = ppc.tile([128, N], F32)
            psum_c = ppc.tile([128, N], F32)
            for half in range(BLOC):
                tp = (0, 64 * half)
                rhs = hT[:, half * N:(half + 1) * N]
                nc.tensor.matmul(psum_a[64 * half:64 * half + 64, :],
                                 w1aT[:], rhs, start=True, stop=True,
                                 tile_position=tp)
                nc.tensor.matmul(psum_c[64 * half:64 * half + 64, :],
                                 w1bT[:], rhs, start=True, stop=True,
                                 tile_position=tp)

            aT2f = cpool.tile([128, N], F32)   # a^T + b1 (f32: scalar operand)
            cT2 = cpool.tile([128, N], BF16)   # c^T cast to bf16 (streamed)
            nc.vector.tensor_scalar_add(aT2f[:], psum_a[:], b1v[:])
            nc.vector.tensor_copy(cT2[:], psum_c[:])

            for g in range(NG):
                psum_t = ppm.tile([128, N], F32)
                for d in range(DUOS_PER_G):
                    i = g * DUOS_PER_G + d
                    z2 = zpool.tile([128, N], BF16)
                    if d % ACT_STRIDE == 1:
                        nc.scalar.activation(z2[:], cT2[:], Relu,
                                             bias=aT2f[:, i:i + 1], scale=1.0)
                    else:
                        nc.vector.tensor_scalar(z2[:], cT2[:],
                                                aT2f[:, i:i + 1], 0.0,
                                                op0=ADD, op1=MAX)
                    nc.tensor.matmul(psum_t[:],
                                     wbig[:, 126 - 2 * d:254 - 2 * d],
                                     z2[:],
                                     start=(d == 0), stop=(d == DUOS_PER_G - 1))
                sig = spool.tile([128, N], F32)
                nc.scalar.activation(sig[:], psum_t[:], Sigmoid,
                                     bias=b2v[:], scale=1.0)
                nc.sync.dma_start(out_d[128 * g:128 * (g + 1), :], sig[:])

    nc.compile()
    return nc


def _get_compiled():
    global _COMPILED
    if _COMPILED is None:
        _COMPILED = _build()
    return _COMPILED


def _prep_in_maps(hidden_state, w1, b1, w2, b2):
    hidden_state = np.asarray(hidden_state, dtype=np.float32)
    w1 = np.asarray(w1, dtype=np.float32)
    b1 = np.asarray(b1, dtype=np.float32)
    w2 = np.asarray(w2, dtype=np.float32)
    b2 = np.asarray(b2, dtype=np.float32)

    w1aT = np.ascontiguousarray(w1[:, :F].T)          # [F, H]
    w1bT = np.ascontiguousarray(w1[:, F:].T)          # [F, H]
    b1v = np.tile(b1, 2).reshape(2 * H, 1)
    b2v = np.full((2 * H, 1), b2[0], dtype=np.float32)
    wbig = np.zeros((128, 256), dtype=ml_dtypes.bfloat16)
    wbig[0:64, 126] = w2[0].astype(ml_dtypes.bfloat16)
    wbig[64:128, 127] = w2[0].astype(ml_dtypes.bfloat16)

    in_maps = []
    for k in range(NCORES):
        shard = hidden_state[BLOC * k:BLOC * (k + 1)]      # [2, 256, 64]
        hTk = np.ascontiguousarray(shard.reshape(IL, F).T)  # [64, 512]
        in_maps.append({
            "hT": hTk, "w1aT": w1aT, "w1bT": w1bT,
            "b1v": b1v, "b2v": b2v, "wbig": wbig,
        })
    return in_maps


def kernel(hidden_state, w1, b1, w2, b2):
    nc = _get_compiled()
    in_maps = _prep_in_maps(hidden_state, w1, b1, w2, b2)
    res = run_bass_kernel_spmd(nc, in_maps, core_ids=list(range(NCORES)))
    out = np.empty((B, N, N), dtype=np.float32)
    for k in range(NCORES):
        flat = res.results[k]["out"]                   # [512, 256] rows = 2i+b
        out[BLOC * k:BLOC * (k + 1)] = (
            flat.reshape(N, BLOC, N).transpose(1, 0, 2))
    idx = np.arange(N)
    out[:, idx, idx] = 1.0
    return out
